# revision 41
# baseline (speedup 1.0000x reference)
"""Trainium2 Bass kernel for a single-step attention GRU decoder.

Problem shapes (hardcoded):
  V=512, E=256, H=256, EMB=256, B=64, S=4096, 8 NeuronCores.

Math:
  scores[s,b] = enc[b,s,:]@w_e + h[b,:]@w_h + attn_b          # [S,B]
  sm = softmax(scores, axis=1)  (over b!)                      # [S,B]
  aw = sm.reshape(B,S)  (contiguous reinterpret)               # [B,S]
  ctx[b,:] = sum_s aw[b,s]*enc[b,s,:]                          # [B,E]
  x = relu([ctx | emb[idx]]); GRU step; logits = log_softmax(dense(h_new))

Sharding: core c owns s-positions [512c,512c+512) for phase A (scores+softmax)
and batches [8c,8c+8) for phase B (ctx) + tail.  Key fact: sm row r only
depends on scores[r,:] (softmax axis is batch), and aw rows for batches
[8c,8c+8) come exactly from sm rows [512c,512c+512) - so there is NO
cross-core communication at all.

Pipeline: 4 position-chunks of 128 rows; chunk pc's softmax unlocks the full
ctx chains of batches {2pc, 2pc+1} (PE) while the next chunk's scores stream.
Everything in the tail that doesn't depend on ctx (x_emb gather, gh chains,
the emb half of the gi chains, activation tables) is hoisted before the loop.
"""

import numpy as np

V, E, H, EMB = 512, 256, 256, 256
B, S = 64, 4096
NC = 8
BL = B // NC      # 8 local batches (phase B / tail)
SL = S // NC      # 512 local s-positions (phase A)

_CACHE = {}


def _build_nc(debug=False):
    import concourse.bass as bass
    from concourse import bacc, mybir
    from concourse.tile import TileContext

    f32 = mybir.dt.float32
    bf16 = mybir.dt.bfloat16
    AF = mybir.ActivationFunctionType
    OP = mybir.AluOpType

    nc = bacc.Bacc(trn_type="TRN2")

    # ---------------- per-core DRAM inputs ----------------
    # enc streams in bf16 (halves HBM traffic; scores/ctx accumulate f32)
    enc_s = nc.dram_tensor("enc_s", [B, SL, E], bf16, kind="ExternalInput")
    # enc_bp: enc[8c:8c+8, :, :] permuted to [bl, p, n, e] with s = 128n + p
    enc_bp = nc.dram_tensor("enc_bp", [BL, 128, S // 128, E], bf16, kind="ExternalInput")
    h_full = nc.dram_tensor("h_full", [B, H], f32, kind="ExternalInput")
    # h_locT[a, p, bl] = h[8c+bl, 128a+p]
    h_locT = nc.dram_tensor("h_locT", [2, 128, BL], f32, kind="ExternalInput")
    idx_loc = nc.dram_tensor("idx_loc", [1, BL], f32, kind="ExternalInput")
    emb_w = nc.dram_tensor("emb_w", [V, EMB], f32, kind="ExternalInput")
    w_e8 = nc.dram_tensor("w_e8", [1, 8 * E], bf16, kind="ExternalInput")
    # w_h_col[k, a] = w_h[128a + k]
    w_h_col = nc.dram_tensor("w_h_col", [128, 2], f32, kind="ExternalInput")
    attn_b = nc.dram_tensor("attn_b", [1, 1], f32, kind="ExternalInput")
    # w_ihT = w_ih.T  [512, 768];  w_hhT = w_hh.T [256, 768]
    w_ihT = nc.dram_tensor("w_ihT", [E + EMB, 3 * H], f32, kind="ExternalInput")
    w_hhT = nc.dram_tensor("w_hhT", [H, 3 * H], f32, kind="ExternalInput")
    # biases in T layout: [p, jc] = bias[128*jc + p]
    b_ihT = nc.dram_tensor("b_ihT", [128, 6], f32, kind="ExternalInput")
    b_hhT = nc.dram_tensor("b_hhT", [128, 6], f32, kind="ExternalInput")
    dense_wT = nc.dram_tensor("dense_wT", [H, V], f32, kind="ExternalInput")
    dense_b = nc.dram_tensor("dense_b", [1, V], f32, kind="ExternalInput")
    ident_in = nc.dram_tensor("ident", [128, 128], f32, kind="ExternalInput")
    # iota4[p, v] = p + 128v
    iota4 = nc.dram_tensor("iota4", [128, 4], f32, kind="ExternalInput")
    # internal DRAM scratch for the hw row round-trip (partition-broadcast
    # DMA needs a DRAM source)
    hw_dram = nc.dram_tensor("hw_scratch", [1, B], f32, kind="Internal")

    # ---------------- per-core DRAM outputs ----------------
    logits_o = nc.dram_tensor("logits", [BL, V], f32, kind="ExternalOutput")
    hnew_o = nc.dram_tensor("h_new", [BL, H], f32, kind="ExternalOutput")
    aw_o = nc.dram_tensor("aw", [BL, S], f32, kind="ExternalOutput")
    if debug:
        dbg_xt = nc.dram_tensor("dbg_xt", [128, 4 * BL], f32, kind="ExternalOutput")
        dbg_hnT = nc.dram_tensor("dbg_hnT", [128, 2 * BL], f32, kind="ExternalOutput")

    # A-group size: batches per phase-A DMA / compute unit
    GA = 8
    NGA = B // GA            # 8 groups per chunk
    # group -> engine: reduce on ACT for g % 4 == 3, else DVE;
    # mul on GPSIMD for odd g, else DVE
    act_groups = [g for g in range(NGA) if g % 4 == 3]           # [3, 7]
    dve_groups = [g for g in range(NGA) if g % 4 != 3]
    # column layout: scores_dve packs dve_groups in order, scores_act likewise
    dve_off = {g: 8 * i for i, g in enumerate(dve_groups)}
    act_off = {g: 8 * i for i, g in enumerate(act_groups)}

    with TileContext(nc) as tc:
        with (
            tc.tile_pool(name="singles", bufs=1) as singles,
            tc.tile_pool(name="enca", bufs=6) as enca,
            tc.tile_pool(name="encb", bufs=4) as encb,
            tc.tile_pool(name="scratch", bufs=4) as scratch,
            tc.tile_pool(name="sm_pool", bufs=2) as sm_pool,
            tc.tile_pool(name="tail", bufs=1) as tail,
            tc.tile_pool(name="ps_tr", bufs=2, space="PSUM") as ps_tr,
            tc.tile_pool(name="ps_acc", bufs=1, space="PSUM") as ps_acc,
        ):
            # ---------- constants into SBUF (gpsimd SWDGE ring) ----------
            w_e_b8 = singles.tile([128, 8 * E], bf16)    # w_e x8, bcast to parts
            nc.gpsimd.dma_start(out=w_e_b8, in_=w_e8.ap().to_broadcast([128, 8 * E]))
            ident = singles.tile([128, 128], f32)
            nc.gpsimd.dma_start(out=ident, in_=ident_in.ap())
            h_sb = singles.tile([B, H], f32)
            nc.gpsimd.dma_start(out=h_sb, in_=h_full.ap())
            whc = singles.tile([128, 2], f32)
            nc.gpsimd.dma_start(out=whc, in_=w_h_col.ap())
            attnb_sb = singles.tile([1, 1], f32)
            nc.gpsimd.dma_start(out=attnb_sb, in_=attn_b.ap())

            # preload all ACT function tables with tiny dummy ops so the
            # table loads don't stall the tail
            warm = singles.tile([1, 4], f32)
            nc.vector.memset(warm, 0.0)
            for fn in (AF.Exp, AF.Sigmoid, AF.Tanh, AF.Ln):
                nc.scalar.activation(out=warm[:, 0:1], in_=warm[:, 0:1], func=fn)

            # ---------- hw row: hw[b] = h[b,:]@w_h + attn_b ----------
            hT = []
            for a in range(2):
                tps = ps_tr.tile([128, 64], f32, tag="tr")
                nc.tensor.transpose(
                    out=tps, in_=h_sb[:, 128 * a:128 * (a + 1)],
                    identity=ident[0:B, 0:B],
                )
                t_sb = singles.tile([128, 64], f32, tag=f"hT{a}")
                nc.scalar.activation(out=t_sb, in_=tps, func=AF.Copy)
                hT.append(t_sb)
            hw_ps = ps_tr.tile([1, B], f32, tag="tr")
            for a in range(2):
                nc.tensor.matmul(
                    hw_ps, lhsT=whc[:, a:a + 1], rhs=hT[a],
                    start=(a == 0), stop=(a == 1),
                )
            hw_row = singles.tile([1, B], f32)
            nc.scalar.activation(
                out=hw_row, in_=hw_ps, func=AF.Identity, bias=attnb_sb, scale=1.0
            )
            nc.gpsimd.dma_start(out=hw_dram.ap(), in_=hw_row)
            # hwb_perm: hw broadcast to 128 partitions, with columns permuted
            # into (dve_groups | act_groups) order
            hwb_perm = singles.tile([128, B], f32)
            pos = 0
            for g in dve_groups + act_groups:
                nc.gpsimd.dma_start(
                    out=hwb_perm[:, pos:pos + GA],
                    in_=hw_dram.ap()[:, GA * g:GA * (g + 1)].to_broadcast([128, GA]),
                )
                pos += GA

            # ---------- early tail work (independent of ctx) ----------
            # x_embT via one-hot matmul gather
            idx_b = tail.tile([128, BL], f32)
            nc.gpsimd.dma_start(out=idx_b, in_=idx_loc.ap().to_broadcast([128, BL]))
            io4 = tail.tile([128, 4], f32)
            nc.gpsimd.dma_start(out=io4, in_=iota4.ap())
            # xt_ps holds [ctx half0, ctx half1, emb half0, emb half1] columns
            xt_ps = ps_acc.tile([128, 4, BL], f32, tag="xt")
            xt2 = xt_ps[:].rearrange("p k bl -> p (k bl)")
            embs, ohs = [], []
            for v in range(4):
                emb_sb = tail.tile([128, EMB], f32, tag=f"emb{v}")
                nc.gpsimd.dma_start(out=emb_sb, in_=emb_w.ap()[128 * v:128 * (v + 1), :])
                embs.append(emb_sb)
                oh = tail.tile([128, BL], f32, tag=f"oh{v}")
                nc.vector.tensor_scalar(
                    out=oh, in0=idx_b, scalar1=io4[:, v:v + 1], scalar2=None,
                    op0=OP.is_equal,
                )
                ohs.append(oh)
            for hh in range(2):
                for v in range(4):
                    nc.tensor.matmul(
                        xt2[:, (2 + hh) * BL:(3 + hh) * BL],
                        lhsT=embs[v][:, 128 * hh:128 * (hh + 1)],
                        rhs=ohs[v],
                        start=(v == 0),
                        stop=(v == 3),
                    )
            xT_all = tail.tile([128, 4, BL], f32)
            nc.scalar.activation(out=xT_all[:, 2:4, :], in_=xt_ps[:, 2:4, :], func=AF.Relu)

            # weights/biases for the GRU + dense
            wih_sb = []
            for kc in range(4):
                t = tail.tile([128, 3 * H], f32, tag=f"wih{kc}")
                nc.gpsimd.dma_start(out=t, in_=w_ihT.ap()[128 * kc:128 * (kc + 1), :])
                wih_sb.append(t)
            whh_sb = []
            for kc in range(2):
                t = tail.tile([128, 3 * H], f32, tag=f"whh{kc}")
                nc.gpsimd.dma_start(out=t, in_=w_hhT.ap()[128 * kc:128 * (kc + 1), :])
                whh_sb.append(t)
            hlT = tail.tile([128, 2, BL], f32)
            nc.gpsimd.dma_start(out=hlT, in_=h_locT.ap().rearrange("a p bl -> p a bl"))
            bihT_sb = tail.tile([128, 6], f32)
            nc.gpsimd.dma_start(out=bihT_sb, in_=b_ihT.ap())
            bhhT_sb = tail.tile([128, 6], f32)
            nc.gpsimd.dma_start(out=bhhT_sb, in_=b_hhT.ap())
            bsumT = tail.tile([128, 6], f32)
            nc.vector.tensor_add(bsumT, bihT_sb, bhhT_sb)
            dw_sb = []
            for kc in range(2):
                t = tail.tile([128, V], f32, tag=f"dw{kc}")
                nc.gpsimd.dma_start(out=t, in_=dense_wT.ap()[128 * kc:128 * (kc + 1), :])
                dw_sb.append(t)
            db_b = tail.tile([BL, V], f32)
            nc.gpsimd.dma_start(out=db_b, in_=dense_b.ap().to_broadcast([BL, V]))

            # gh chains (need only hlT) and the emb half of the gi chains
            ghT_ps = ps_acc.tile([128, 6, BL], f32, tag="ghT")
            for jc in range(6):
                for kc in range(2):
                    nc.tensor.matmul(
                        ghT_ps[:, jc, :],
                        lhsT=whh_sb[kc][:, 128 * jc:128 * (jc + 1)],
                        rhs=hlT[:, kc, :],
                        start=(kc == 0),
                        stop=(kc == 1),
                    )
            gi_emb_ps = ps_acc.tile([128, 6, BL], f32, tag="gi_emb")
            for jc in range(6):
                for kc in (2, 3):
                    nc.tensor.matmul(
                        gi_emb_ps[:, jc, :],
                        lhsT=wih_sb[kc][:, 128 * jc:128 * (jc + 1)],
                        rhs=xT_all[:, kc, :],
                        start=(kc == 2),
                        stop=(kc == 3),
                    )
            # move the parts of gh/gi_emb needed later out of PSUM early
            gh_rz_sb = tail.tile([128, 4, BL], f32)
            nc.scalar.activation(out=gh_rz_sb, in_=ghT_ps[:, 0:4, :], func=AF.Copy)
            ghnT = tail.tile([128, 2, BL], f32)
            ginT_a = tail.tile([128, 2, BL], f32)
            for jc in (4, 5):
                nc.scalar.activation(
                    out=ghnT[:, jc - 4, :], in_=ghT_ps[:, jc, :], func=AF.Identity,
                    bias=bhhT_sb[:, jc:jc + 1], scale=1.0,
                )
                nc.scalar.activation(
                    out=ginT_a[:, jc - 4, :], in_=gi_emb_ps[:, jc, :], func=AF.Identity,
                    bias=bihT_sb[:, jc:jc + 1], scale=1.0,
                )

            # ---------- pipelined phases A+B over 4 position-chunks ----------
            # NOTE: PSUM accumulation chains must NOT interleave with another
            # chain's start=True in the same bank (observed on HW: a start
            # clears the whole bank's has_written bits, dropping the sibling
            # chain's first contribution).  Keep each chain contiguous.
            ctx_row_sb = tail.tile([1, BL * E], f32)
            for pc in range(4):
                # -- scores for rows r = 128pc + p, all 64 batches, in groups
                #    of GA=8 batches per DMA / compute op --
                scores_dve = sm_pool.tile([128, 8 * len(dve_groups)], f32, tag="sc_d")
                scores_act = sm_pool.tile([128, 8 * len(act_groups)], f32, tag="sc_a")
                for g in range(NGA):
                    et = enca.tile([128, GA, E], bf16, tag="enca")
                    nc.sync.dma_start(
                        out=et,
                        in_=enc_s.ap()[GA * g:GA * (g + 1),
                                       128 * pc:128 * (pc + 1), :]
                        .rearrange("b p e -> p b e"),
                    )
                    et2 = et[:].rearrange("p b e -> p (b e)")
                    if g % 2 == 1:
                        sc = scratch.tile([128, GA, E], bf16, tag="mul_g")
                        nc.gpsimd.tensor_mul(
                            sc[:].rearrange("p b e -> p (b e)"), et2, w_e_b8
                        )
                    else:
                        sc = scratch.tile([128, GA, E], bf16, tag="mul_v")
                        nc.vector.tensor_mul(
                            sc[:].rearrange("p b e -> p (b e)"), et2, w_e_b8
                        )
                    if g in act_groups:
                        for j in range(GA):
                            nc.scalar.activation(
                                out=sc[:, j, :], in_=sc[:, j, :], func=AF.Identity,
                                accum_out=scores_act[:, act_off[g] + j:
                                                     act_off[g] + j + 1],
                            )
                    else:
                        nc.vector.reduce_sum(
                            out=scores_dve[:, dve_off[g]:dve_off[g] + GA],
                            in_=sc, axis=mybir.AxisListType.X,
                        )

                # -- softmax over b on the split tiles --
                nd = 8 * len(dve_groups)
                nc.vector.tensor_add(scores_dve, scores_dve, hwb_perm[:, 0:nd])
                nc.vector.tensor_add(scores_act, scores_act, hwb_perm[:, nd:B])
                mx_e = sm_pool.tile([128, 1], f32, tag="mx_e")
                mx_o = sm_pool.tile([128, 1], f32, tag="mx_o")
                nc.vector.reduce_max(out=mx_e, in_=scores_dve, axis=mybir.AxisListType.X)
                nc.vector.reduce_max(out=mx_o, in_=scores_act, axis=mybir.AxisListType.X)
                mx = sm_pool.tile([128, 1], f32, tag="mx")
                nc.vector.tensor_max(mx, mx_e, mx_o)
                nc.vector.tensor_scalar_sub(scores_dve, scores_dve, mx)
                nc.vector.tensor_scalar_sub(scores_act, scores_act, mx)
                se_e = sm_pool.tile([128, 1], f32, tag="se_e")
                se_o = sm_pool.tile([128, 1], f32, tag="se_o")
                ex_e = sm_pool.tile([128, 8 * len(dve_groups)], f32, tag="ex_e")
                ex_o = sm_pool.tile([128, 8 * len(act_groups)], f32, tag="ex_o")
                nc.scalar.activation(out=ex_e, in_=scores_dve, func=AF.Exp, accum_out=se_e)
                nc.scalar.activation(out=ex_o, in_=scores_act, func=AF.Exp, accum_out=se_o)
                se = sm_pool.tile([128, 1], f32, tag="se")
                nc.vector.tensor_add(se, se_e, se_o)
                rc = sm_pool.tile([128, 1], f32, tag="rc")
                nc.vector.reciprocal(out=rc, in_=se)
                smd = sm_pool.tile([128, 8 * len(dve_groups)], f32, tag="smd")
                sma = sm_pool.tile([128, 8 * len(act_groups)], f32, tag="sma")
                nc.vector.tensor_scalar_mul(smd, ex_e, rc)
                nc.vector.tensor_scalar_mul(sma, ex_o, rc)
                # recombine into batch order
                sm_pc = sm_pool.tile([128, B], f32, tag="sm_pc")
                nc.vector.tensor_copy(sm_pc[:, 0:24], smd[:, 0:24])
                nc.vector.tensor_copy(sm_pc[:, 32:56], smd[:, 24:48])
                nc.vector.tensor_copy(sm_pc[:, 24:32], sma[:, 0:8])
                nc.vector.tensor_copy(sm_pc[:, 56:64], sma[:, 8:16])

                # aw rows for batches 2pc, 2pc+1 (contiguous partition-major)
                nc.sync.dma_start(
                    out=aw_o.ap()[2 * pc:2 * pc + 2].rearrange(
                        "bl (x y) -> (bl x) y", x=B, y=B
                    ),
                    in_=sm_pc,
                )

                # -- stitch aw columns for the PE lhsT:
                #    smT2[j*64+k, 32*blh + n] = sm_pc[64*blh + 2n + j, k] --
                smTp = ps_tr.tile([B, 128], f32, tag="tr")
                nc.tensor.transpose(out=smTp, in_=sm_pc, identity=ident)
                smT2 = sm_pool.tile([128, B], bf16, tag="smT2")
                smTp_v = smTp[:].rearrange("k (g n two) -> k g n two", g=2, two=2)
                for j in range(2):
                    nc.vector.tensor_copy(
                        smT2[B * j:B * (j + 1), :].rearrange(
                            "k (g n) -> k g n", g=2
                        ),
                        smTp_v[:, :, :, j],
                    )

                # -- ctx chains for batches 2pc, 2pc+1 --
                for blh in range(2):
                    bl = 2 * pc + blh
                    bts = []
                    for j in range(2):  # two 1MB chunks of 16 n-blocks
                        bt = encb.tile([128, 16, E], bf16, tag=f"encb{j}")
                        nc.scalar.dma_start(
                            out=bt, in_=enc_bp.ap()[bl][:, 16 * j:16 * (j + 1), :]
                        )
                        bts.append(bt)
                    crow = ps_acc.tile([1, E], f32, tag="crow")
                    for n in range(32):
                        nc.tensor.matmul(
                            crow,
                            lhsT=smT2[:, 32 * blh + n:32 * blh + n + 1],
                            rhs=bts[n // 16][:, n % 16, :],
                            start=(n == 0),
                            stop=(n == 31),
                        )
                    nc.scalar.activation(
                        out=ctx_row_sb[:, bl * E:(bl + 1) * E], in_=crow, func=AF.Copy
                    )

            # ---------- late tail: ctx transposes -> gates -> outputs -------
            for bl in range(BL):
                for a in range(2):
                    nc.tensor.transpose(
                        out=xt2[:, a * BL + bl:a * BL + bl + 1],
                        in_=ctx_row_sb[:, bl * E + 128 * a:bl * E + 128 * (a + 1)],
                        identity=ident[0:1, 0:1],
                    )
            nc.scalar.activation(out=xT_all[:, 0:2, :], in_=xt_ps[:, 0:2, :], func=AF.Relu)
            if debug:
                dbgx = tail.tile([128, 4, BL], f32)
                nc.vector.tensor_copy(dbgx, xt_ps)
                nc.sync.dma_start(
                    out=dbg_xt.ap().rearrange("p (k bl) -> p k bl", k=4), in_=dbgx
                )

            # ctx half of gi chains
            gi_ctx_ps = ps_acc.tile([128, 6, BL], f32, tag="gi_ctx")
            for jc in range(6):
                for kc in (0, 1):
                    nc.tensor.matmul(
                        gi_ctx_ps[:, jc, :],
                        lhsT=wih_sb[kc][:, 128 * jc:128 * (jc + 1)],
                        rhs=xT_all[:, kc, :],
                        start=(kc == 0),
                        stop=(kc == 1),
                    )

            # r,z gates: sigmoid(gi_emb + gi_ctx + gh + b_ih + b_hh)
            sum1 = tail.tile([128, 4, BL], f32)
            nc.vector.tensor_add(sum1, gi_emb_ps[:, 0:4, :], gh_rz_sb)
            sum_rz = tail.tile([128, 4, BL], f32)
            nc.vector.tensor_add(sum_rz, gi_ctx_ps[:, 0:4, :], sum1)
            rzT = tail.tile([128, 4, BL], f32)
            for jc in range(4):
                nc.scalar.activation(
                    out=rzT[:, jc, :], in_=sum_rz[:, jc, :], func=AF.Sigmoid,
                    bias=bsumT[:, jc:jc + 1], scale=1.0,
                )
            # n gate: tanh(i_n + b_ih_n + r*(h@w_hh_n + b_hh_n))
            ginT = tail.tile([128, 2, BL], f32)
            nc.vector.tensor_add(ginT, gi_ctx_ps[:, 4:6, :], ginT_a)
            rhn = tail.tile([128, 2, BL], f32)
            nc.vector.tensor_mul(rhn, rzT[:, 0:2, :], ghnT)
            pre_n = tail.tile([128, 2, BL], f32)
            nc.vector.tensor_add(pre_n, ginT, rhn)
            nT = tail.tile([128, 2, BL], f32)
            nc.scalar.activation(out=nT, in_=pre_n, func=AF.Tanh)
            # h_new = n + z*(h - n)
            hmn = tail.tile([128, 2, BL], f32)
            nc.vector.tensor_sub(hmn, hlT, nT)
            zhmn = tail.tile([128, 2, BL], f32)
            nc.vector.tensor_mul(zhmn, rzT[:, 2:4, :], hmn)
            hnT = tail.tile([128, 2, BL], f32)
            nc.vector.tensor_add(hnT, nT, zhmn)
            if debug:
                nc.sync.dma_start(
                    out=dbg_hnT.ap().rearrange("p (k bl) -> p k bl", k=2), in_=hnT
                )

            # h_new output: transpose [128, 16] -> [16, 128], then 2 row DMAs
            hn_tr = ps_tr.tile([16, 128], f32, tag="tr")
            nc.tensor.transpose(
                out=hn_tr, in_=hnT[:].rearrange("p a bl -> p (a bl)"), identity=ident
            )
            hn_sb = tail.tile([16, 128], f32)
            nc.scalar.activation(out=hn_sb, in_=hn_tr, func=AF.Copy)
            for a in range(2):
                nc.sync.dma_start(
                    out=hnew_o.ap()[:, 128 * a:128 * (a + 1)],
                    in_=hn_sb[8 * a:8 * (a + 1), :],
                )

            # ---------- dense + log_softmax ----------
            lg_ps = ps_tr.tile([BL, V], f32, tag="tr")
            for kc in range(2):
                nc.tensor.matmul(
                    lg_ps, lhsT=hnT[:, kc, :], rhs=dw_sb[kc],
                    start=(kc == 0), stop=(kc == 1),
                )
            lg = tail.tile([BL, V], f32)
            nc.vector.tensor_add(lg, lg_ps, db_b)
            mx2 = tail.tile([BL, 1], f32)
            nc.vector.reduce_max(out=mx2, in_=lg, axis=mybir.AxisListType.X)
            nc.vector.tensor_scalar_sub(lg, lg, mx2)
            elg = tail.tile([BL, V], f32)
            se2 = tail.tile([BL, 1], f32)
            nc.scalar.activation(out=elg, in_=lg, func=AF.Exp, accum_out=se2)
            lse = tail.tile([BL, 1], f32)
            nc.scalar.activation(out=lse, in_=se2, func=AF.Ln)
            logits_sb = tail.tile([BL, V], f32)
            nc.vector.tensor_scalar_sub(logits_sb, lg, lse)
            nc.sync.dma_start(out=logits_o.ap(), in_=logits_sb)

    nc.finalize()
    return nc


def _get_nc(debug=False):
    key = ("nc", debug)
    if key not in _CACHE:
        _CACHE[key] = _build_nc(debug)
    return _CACHE[key]


def _prep_in_maps(decoder_input, current_hidden_state, encoder_outputs, emb,
                  attn_w, attn_b, w_ih, w_hh, b_ih, b_hh, dense_w, dense_b):
    import ml_dtypes
    f32 = np.float32
    bf16 = ml_dtypes.bfloat16
    dec = np.asarray(decoder_input).astype(f32).reshape(B)
    h = np.ascontiguousarray(np.asarray(current_hidden_state, f32)[0])      # [B, H]
    enc = np.asarray(encoder_outputs, f32).astype(bf16)                      # [B, S, E]
    emb = np.ascontiguousarray(np.asarray(emb, f32))
    w = np.asarray(attn_w, f32)[0]
    w_h, w_e = w[:H], w[H:]
    shared = {
        "h_full": h,
        "emb_w": emb,
        "w_e8": np.ascontiguousarray(np.tile(w_e, 8).reshape(1, 8 * E)).astype(bf16),
        "w_h_col": np.ascontiguousarray(w_h.reshape(2, 128).T),
        "attn_b": np.asarray(attn_b, f32).reshape(1, 1),
        "w_ihT": np.ascontiguousarray(np.asarray(w_ih, f32).T),
        "w_hhT": np.ascontiguousarray(np.asarray(w_hh, f32).T),
        "b_ihT": np.ascontiguousarray(np.asarray(b_ih, f32).reshape(6, 128).T),
        "b_hhT": np.ascontiguousarray(np.asarray(b_hh, f32).reshape(6, 128).T),
        "dense_wT": np.ascontiguousarray(np.asarray(dense_w, f32).T),
        "dense_b": np.ascontiguousarray(np.asarray(dense_b, f32).reshape(1, V)),
        "ident": np.eye(128, dtype=f32),
        "iota4": np.ascontiguousarray(
            (np.arange(128, dtype=f32)[:, None] + 128.0 * np.arange(4, dtype=f32))
        ),
    }
    in_maps = []
    for c in range(NC):
        bsl = slice(BL * c, BL * (c + 1))
        enc_b = enc[bsl]                                        # [8, 4096, 256]
        in_maps.append({
            **shared,
            "enc_s": np.ascontiguousarray(enc[:, SL * c:SL * (c + 1), :]),
            "enc_bp": np.ascontiguousarray(
                enc_b.reshape(BL, S // 128, 128, E).transpose(0, 2, 1, 3)
            ),
            "h_locT": np.ascontiguousarray(
                h[bsl].T.reshape(2, 128, BL, order="C")
            ),
            "idx_loc": np.ascontiguousarray(dec[bsl].reshape(1, BL)),
        })
    return in_maps


def run(inputs, trace=False, debug=False, **kwargs):
    """Compile (cached) + run on 8 cores; returns BassKernelResults."""
    from concourse import bass_utils
    nc = _get_nc(debug)
    in_maps = _prep_in_maps(**inputs)
    res = bass_utils.run_bass_kernel_spmd(
        nc, in_maps, core_ids=list(range(NC)), trace=trace, **kwargs
    )
    return res


def kernel(**inputs):
    res = run(inputs)
    outs = res.results
    logits = np.concatenate([o["logits"] for o in outs], axis=0)
    h_new = np.concatenate([o["h_new"] for o in outs], axis=0)[None]
    aw = np.concatenate([o["aw"] for o in outs], axis=0)
    return logits, h_new, aw
